# revision 39
# baseline (speedup 1.0000x reference)
"""Trainium2 Bass kernel for nn_Def_A2C_Sample_Generator.

Computation (see reference):
  x = concat(state, payoff, noise)            (500, 504)
  h1 = lrelu(bn(adj @ (x @ w1) + b1))         (500, 32)
  h2 = lrelu(bn(adj @ (h1 @ w2) + b2))        (500, 16)
  xf = h2.reshape(8000)
  logits = xf @ actgen_w + def_cur_loc @ actgen_v          (50, 500)
  out = softmax(logits[None] + gumbel(u), axis=-1)         (1000, 50, 500)

Sharding: data-parallel over the 1000 samples, 125 per core on 8
cores; actgen_w is channel-sharded 2-of-16 per core (only the owned 2
h2 channels are computed); the 2KB z partials are joined with an ncfw
AllGather ([1,500] -> [8,500]) + a K=8 f32 ones-matmul into the
logits PSUM (AG floor ~5us vs AllReduce ~10us, and the f32 gather+sum
matches AllReduce numerics).

KEY MEASURED FACTS driving this design (8 traced HW runs):
- Each DMA queue sustains only ~120-130GB/s (either a 5-engine SDMA
  allotment at line rate, or descriptor-emission limits); queues run
  concurrently BUT share the 16 SDMA engines at packet granularity,
  so bulk u streams on other queues STARVE the small param
  descriptors (adjT slipped 13us -> 53us; z trigger 78us). Params
  therefore own the sync-ring head, u follows on the same ring.
- The ncfw collective BARRIER ends at runtime-init-skew time (42 to
  103us across runs!) regardless of when collectives are triggered;
  the first collective starts at barrier_end+11us and runs ~15-26us.
  A warmup collective is NET NEGATIVE (it serializes its own ~15us
  ahead of the z collective and saves nothing). The z AllGather
  completes at barrier_end + ~37us; this jitter dominates run-to-run
  variance and nothing in the kernel can hide it (logits gate all
  per-sample DVE work).
- Engine offloads of the normalize measured: ACT Copy+scale ~1us/r
  (3x DVE), Pool tensor_scalar 7.4us/r AND it degrades concurrent
  DVE ops 12x via SBUF port contention. Everything stays on DVE.
- Emission ORDER is scheduling priority: the logits-gated exp/pack
  emitted before the ACT chunk passes stalls ACT ~45us behind the
  (cost-model-invisible) collective latency; emitted after ALL of
  them, the pack waits for ACT to drain (~130us). It goes mid-queue,
  between chunk 2's and chunk 3's passes.

Softmax factorization keeps all gumbel work independent of logits:
  exp(logits + g) with g = -ln(-ln u) equals L * a where
  L = exp(logits) (prologue row) and a = exp(-ln(-ln u)) = -1/ln u.
Main loop, CH=10 r's per chunk in the natural (sample, r, T) layout:
  a       : 3 chunk-wide ACT passes (Ln, Ln(-x) in-place f32, then
            Exp(-x) into a separate bf16 tile so the u tile recycles
            at ACT pace); one table set via the act-table monkeypatch
  L bcast : per-r PE ones-matmul, single bf16 plane into f32 PSUM
  q, S    : DVE scalar_tensor_tensor mult (bf16 out) + fused row-sum
  out     : one DVE reciprocal per chunk + per-r tensor_scalar mult
            into a bf16 chunk tile, one 1.25MB store per chunk
            (host upcasts to f32; rel err ~5e-3, gate is 2e-2)

u MUST stay f32 (a = -1/ln u amplifies input error by 1/(1-u)).
All matmul operands that tolerate bf16 are bf16. Params are packed
into two planes; the z-sum matmul is f32 (exact partial-sum join).

DMA queues: params (GCN-dependency order) then u chunks 0-3 on the
sync HWDGE ring; wr + u chunk 4 + zin/zag/L-bounces + output stores
on the gpsimd SWDGE ring (idle mid-kernel, so the logits pack fires
the moment the AllGather lands); scalar ring unused for bulk (its
issues stall ACT mid-pass, and mid-kernel its queue sits behind ACT's
pass backlog).

Known-bad variants (all HW-measured, do not retry): Pool-engine
tensor_scalar_mul normalize (7.4us/r + 12x DVE degradation); ACT
normalize offload; ACT-engine PSUM->SBUF bcast copies before the stt
(device UNRECOVERABLE); 2-pass Ln+Reciprocal (no act table has both;
nc.scalar Reciprocal banned); warmup collective; u spread across
scalar/gpsimd ahead of params; replicated-z without the collective
(full xf@actgen_w is 64 N=500 PE matmuls = ~40us serial).
"""
import sys

if "/opt/trn_rl_repo" not in sys.path:
    sys.path.insert(0, "/opt/trn_rl_repo")

import numpy as np

import concourse.bacc as bacc
import concourse.bass as bass
import concourse.mybir as mybir
import concourse.tile as tile
from concourse import bass_utils

# The act-table-load pass resolves Exp -> exp_and_others (id 0) and
# Ln -> natural_log (id 5), thrashing a ~2.7us table swap at every
# Ln<->Exp transition in the main loop. natural_log_exp_and_others
# (id 6) holds BOTH. Hide exp/ln from the other sets in the map the
# chooser reads (ids keep indexing the real act_info.json, so the
# loaded tables are unchanged) so every Exp and Ln lands on set 6 and
# one load suffices.
_orig_get_act_tables = bacc.get_activation_tables


def _patched_get_act_tables(arch):
    tabs = dict(_orig_get_act_tables(arch))
    both = {mybir.ActivationFunctionType.Exp, mybir.ActivationFunctionType.Ln}
    for name, fns in tabs.items():
        if name != "natural_log_exp_and_others" and (both & fns):
            tabs[name] = fns - both
    return tabs


bacc.get_activation_tables = _patched_get_act_tables

F32 = mybir.dt.float32
BF16 = mybir.dt.bfloat16
NCORES = 8
T = 500
R = 50
NS = 1000
SP = NS // NCORES  # 125 samples per core
H1, H2 = 32, 16
FIN = 504  # 2 + 500 + 2 input features
KT = 4  # K/M tiling of the 500 dim into 4x125
NEG_SLOPE = 0.2
CH = 10  # r's per chunk: 20KB per-partition DMA descriptors
NCH = R // CH

_CACHE = {}


def _build():
    nc = bacc.Bacc("TRN2", target_bir_lowering=False, debug=False,
                   enable_asserts=False, num_devices=NCORES)

    # ---- I/O ----
    din = {}
    # pbf[p, :] = adjT k-tiles (4x500) | av k-tiles (4x500) | dclT (4x50)
    din["pbf"] = nc.dram_tensor("pbf", [125, 8 * T + 4 * R], BF16,
                                kind="ExternalInput")
    # pxb[p, :] = xT k-tiles (4x500) | w1 k-tiles (4x32), bf16
    din["pxb"] = nc.dram_tensor("pxb", [126, 4 * T + 4 * H1], BF16,
                                kind="ExternalInput")
    # rows[0, :] = b1 (32) | b2sel (2) | grow (500) | brow (500)
    din["rows"] = nc.dram_tensor("rows", [1, H1 + 2 + 2 * T], BF16,
                                 kind="ExternalInput")
    # only the 2 owned output channels of gc2
    din["w2"] = nc.dram_tensor("w2", [H1, 2], F32, kind="ExternalInput")
    # per-core actgen_w shard: 2 of 16 channels, packed [p][c][k*T+t]
    # so one DMA moves 16KB contiguous per partition
    din["wr"] = nc.dram_tensor("wr", [125, 2 * KT * T], BF16,
                               kind="ExternalInput")
    # one-hot broadcast stationaries: oh[k, r*128+s] = (k == r)
    din["oh"] = nc.dram_tensor("oh", [R, R * 128], BF16,
                               kind="ExternalInput")
    din["u"] = nc.dram_tensor("u", [SP, R, T], F32, kind="ExternalInput")
    out = nc.dram_tensor("out", [SP, R, T], BF16, kind="ExternalOutput")

    with tile.TileContext(nc) as tc:
        _emit(nc, tc, din, out)
    nc.compile()
    return nc


def _emit(nc, tc, din, out):
    from contextlib import ExitStack

    ctx = ExitStack()
    with ctx:
        # ---------- pools ----------
        const = ctx.enter_context(tc.tile_pool(name="const", bufs=1))
        small = ctx.enter_context(tc.tile_pool(name="small", bufs=1))
        psum = ctx.enter_context(tc.tile_pool(name="psum", bufs=1, space="PSUM"))
        dram = ctx.enter_context(tc.tile_pool(name="dram", bufs=1, space="DRAM"))

        CW = CH * T
        upool = ctx.enter_context(tc.tile_pool(name="upool", bufs=3))
        apool = ctx.enter_context(tc.tile_pool(name="apool", bufs=NCH))
        opool = ctx.enter_context(tc.tile_pool(name="opool", bufs=3))
        spool = ctx.enter_context(tc.tile_pool(name="spool", bufs=2))
        bppool = ctx.enter_context(tc.tile_pool(name="bppool", bufs=5,
                                                space="PSUM"))

        onesb = const.tile([65, 128], BF16, tag="onesb", name="onesb")
        nc.vector.memset(onesb[:], 1.0)
        ones32 = const.tile([8, R], F32, tag="ones32", name="ones32")
        nc.vector.memset(ones32[:], 1.0)

        # (NO warmup collective: measured across 5 runs, the ncfw
        # barrier ends at runtime-init-skew time (42-103us) regardless
        # of when collectives are triggered, and the first collective
        # starts at barrier_end+11us. A warmup therefore only ADDS its
        # own ~15us execution ahead of the z AllGather.)

        # ---------- param loads, all on the sync ring in GCN-dependency
        # order (measured: the sync HWDGE queue runs ~130GB/s flat -- 5
        # SDMA engines at line rate -- and the scalar queue is
        # emission-limited and SLOW for small descriptors, so params
        # belong on sync; bulk u chunks go to the other queues) --------
        pxb = const.tile([126, 4 * T + 4 * H1], BF16, tag="pxb", name="pxb")
        nc.sync.dma_start(pxb[:], din["pxb"][:])
        PHALF = 4 * T  # adjT k-tiles first (unblocks a1ps); av+dclT only
        # feed the early lgv matmuls, so they load after rows/w2
        pbf = const.tile([125, 8 * T + 4 * R], BF16, tag="pbf", name="pbf")
        nc.sync.dma_start(pbf[:, :PHALF], din["pbf"][:, :PHALF])
        rows = const.tile([1, H1 + 2 + 2 * T], BF16, tag="rows", name="rows")
        nc.sync.dma_start(rows[:], din["rows"][:])
        w2 = const.tile([H1, 2], F32, tag="w2", name="w2")
        nc.sync.dma_start(w2[:], din["w2"][:])
        nc.sync.dma_start(pbf[:, PHALF:], din["pbf"][:, PHALF:])
        oh = const.tile([R, R * 128], BF16, tag="oh", name="oh")
        nc.sync.dma_start(oh[:], din["oh"][:])

        adjT = [pbf[:, k * T:(k + 1) * T] for k in range(KT)]
        av = [pbf[:, (KT + k) * T:(KT + k + 1) * T] for k in range(KT)]
        dclT = [pbf[:, 8 * T + k * R:8 * T + (k + 1) * R] for k in range(KT)]
        w1 = [pxb[:, 4 * T + k * H1:4 * T + (k + 1) * H1] for k in range(KT)]
        b1 = rows[0:1, 0:H1]
        b2 = rows[0:1, H1:H1 + 2]
        GB = H1 + 2  # rows-pack offset of grow
        grow = rows[0:1, GB:GB + T]
        brow = rows[0:1, GB + T:GB + 2 * T]

        # per-core wr shard (2 channels, 1MB, one 16KB-per-partition
        # DMA). MUST be emitted before the u chunks that share the
        # gpsimd ring: SWDGE drains in emission order and wr gates the
        # z partial -> collective -> the whole logits-dependent tail.
        wpool = ctx.enter_context(tc.tile_pool(name="wpool", bufs=1))
        wrm = wpool.tile([125, 2 * KT * T], BF16, tag="wr_stream",
                         name="wr_stream")
        nc.gpsimd.dma_start(wrm[:], din["wr"][:])
        wgs = [wrm[:, c * KT * T:(c + 1) * KT * T] for c in range(2)]

        # u stream: chunks 0-3 on the sync ring BEHIND the params,
        # chunk 4 on gpsimd behind wr. Spreading u wider was tried
        # twice and made things WORSE: SDMA engines round-robin between
        # queues at packet granularity, so concurrent big u packets
        # starve the small param descriptors (adjT landed at 53us
        # instead of 13, pushing the z trigger out). One early c4 on
        # the otherwise-idle gpsimd ring pulls the last a-chunk in by
        # ~25us without meaningfully contending the param window.
        # (c0 gets its own 1-buf pool so its gpsimd DMA never waits on
        # an upool slot release -- a slot wait would head-of-line-block
        # the zin bounce queued behind it. c0 rides gpsimd because ACT
        # consumes it FIRST: it lands ~26us vs ~44us behind the params
        # on sync, pulling the whole ACT pipeline earlier.)
        u2pool = ctx.enter_context(tc.tile_pool(name="u2pool", bufs=1))
        uts = []
        for ci in range(NCH):
            if ci == 0:
                ut = u2pool.tile([SP, CW], F32, tag="u0", name="u0")
                nc.gpsimd.dma_start(
                    ut[:].rearrange("p (c t) -> p c t", c=CH),
                    din["u"][:, ci * CH:(ci + 1) * CH, :])
            else:
                ut = upool.tile([SP, CW], F32, tag="u", name="u")
                nc.sync.dma_start(
                    ut[:].rearrange("p (c t) -> p c t", c=CH),
                    din["u"][:, ci * CH:(ci + 1) * CH, :])
            uts.append(ut)

        # ---------- logits av part, accumulated EARLY (independent of
        # z); the PSUM group stays open until the z-sum matmul ----------
        lgp = psum.tile([R, T], F32, tag="ps_lg", name="ps_lg")
        for k in range(KT):
            nc.tensor.matmul(lgp[:], dclT[k], av[k],
                             start=(k == 0), stop=False)

        # ---------- GCN, transposed formulation ----------
        # bn folded into the adjacency host-side (adjT ships
        # gamma[t]*adj[t,u] transposed), leaving rank-1 bias terms.
        # Only the 2 owned h2 channels are computed (w2 ships 2 cols).
        def lrelu_from_psum(ps_ap, out_tile, width):
            tmp = small.tile([width, T], F32, tag=f"lr{width}", name=f"lr{width}")
            nc.vector.tensor_scalar_mul(tmp[:], ps_ap, NEG_SLOPE)
            nc.vector.tensor_tensor(out_tile[:], tmp[:], ps_ap,
                                    op=mybir.AluOpType.max)

        xw1 = [small.tile([125, H1], BF16, tag=f"xw1{m}", name=f"xw1{m}") for m in range(KT)]
        for m in range(KT):
            ps = psum.tile([125, H1], F32, tag="ps_small", name="ps_small")
            for k in range(KT):
                nc.tensor.matmul(ps[:], pxb[:, k * T + m * 125:k * T + (m + 1) * 125],
                                 w1[k], start=(k == 0), stop=(k == KT - 1))
            nc.vector.tensor_copy(xw1[m][:], ps[:])

        a1ps = psum.tile([H1, T], F32, tag="ps_small", name="ps_small")
        for k in range(KT):
            nc.tensor.matmul(a1ps[:], xw1[k][:], adjT[k],
                             start=(k == 0), stop=False)
        nc.tensor.matmul(a1ps[:], b1, grow, start=False, stop=False)
        nc.tensor.matmul(a1ps[:], onesb[0:1, :H1], brow, start=False,
                         stop=True)
        h1T = small.tile([H1, T], F32, tag="h1T", name="h1T")
        lrelu_from_psum(a1ps[:], h1T, H1)

        xw2 = [small.tile([125, 2], BF16, tag=f"xw2{m}", name=f"xw2{m}") for m in range(KT)]
        for m in range(KT):
            ps = psum.tile([125, 2], F32, tag="ps_small", name="ps_small")
            nc.tensor.matmul(ps[:], h1T[:, m * 125:(m + 1) * 125], w2[:],
                             start=True, stop=True)
            nc.vector.tensor_copy(xw2[m][:], ps[:])

        # layer 2, only the 2 owned channels, in [t, c] layout
        h2b = [small.tile([125, 2], BF16, tag=f"h2b{m}", name=f"h2b{m}")
               for m in range(KT)]
        for m in range(KT):
            m0 = m * 125
            pt = psum.tile([125, 2], F32, tag="ps_small", name="ps_small")
            for k in range(KT):
                nc.tensor.matmul(pt[:], pbf[:, k * T + m0:k * T + m0 + 125],
                                 xw2[k][:], start=(k == 0), stop=False)
            nc.tensor.matmul(pt[:], rows[0:1, GB + m0:GB + m0 + 125], b2,
                             start=False, stop=False)
            nc.tensor.matmul(pt[:], rows[0:1, GB + T + m0:GB + T + m0 + 125],
                             onesb[0:1, :2], start=False, stop=True)
            tmp2 = small.tile([125, 2], F32, tag=f"lr2{m}", name=f"lr2{m}")
            nc.vector.tensor_scalar_mul(tmp2[:], pt[:], NEG_SLOPE)
            nc.vector.tensor_tensor(h2b[m][:], tmp2[:], pt[:],
                                    op=mybir.AluOpType.max)

        # ---------- z partial (2 owned channels) + AllGather ----------
        zps = psum.tile([1, T], F32, tag="ps_z", name="ps_z")
        first = True
        for c in range(2):
            wt = wgs[c]
            for k in range(KT):
                nc.tensor.matmul(zps[:], h2b[k][:, c:c + 1],
                                 wt[:, k * T:(k + 1) * T],
                                 start=first,
                                 stop=(c == 1 and k == KT - 1))
                first = False
        zpart = small.tile([1, T], F32, tag="zpart", name="zpart")
        nc.vector.tensor_copy(zpart[:], zps[:])
        zin = dram.tile([1, T], F32, name="zin")
        zag_d = dram.tile([NCORES, T], F32, addr_space="Shared", name="zag_d")
        # bounce DMAs on the gpsimd ring: sync is mid-u-stream (HOL
        # block) and a scalar-ring issue would stall ACT mid-pass
        nc.gpsimd.dma_start(zin[:], zpart[:])
        nc.gpsimd.collective_compute(
            "AllGather", mybir.AluOpType.bypass,
            replica_groups=[list(range(NCORES))],
            ins=[zin.opt()], outs=[zag_d.opt()])
        # ---------- main loop part A: the gumbel a = -1/ln(u) passes.
        # The logits-gated exp/pack is emitted BETWEEN chunk 2's and
        # chunk 3's passes: emission order is the scheduler's static
        # priority, so exp-first stalls ACT ~45us behind the
        # (cost-model-invisible) AllGather, while exp-last delays the
        # pack until ACT drains all passes (~130us). Mid-queue splits
        # the difference for the observed AllGather-landing range.
        ats = []

        def emit_a_passes(ci):
            ut = uts[ci]
            nc.scalar.activation(ut[:], ut[:], mybir.ActivationFunctionType.Ln)
            nc.scalar.activation(ut[:], ut[:], mybir.ActivationFunctionType.Ln,
                                 scale=-1.0)
            at = apool.tile([SP, CW], BF16, tag="a", name="a")
            nc.scalar.activation(at[:], ut[:], mybir.ActivationFunctionType.Exp,
                                 scale=-1.0)
            ats.append(at)

        for ci in range(3):
            emit_a_passes(ci)

        zag = small.tile([NCORES, T], F32, tag="zag", name="zag")
        nc.gpsimd.dma_start(zag[:], zag_d[:])

        # z-sum joined into the logits PSUM with a K=8 f32 ones-matmul
        nc.tensor.matmul(lgp[:], ones32[:, :R], zag[:], start=False, stop=True)

        # L = exp(logits) stays in its natural [50, 500] layout (base
        # partition 0, legal). The old design bounced rows through DRAM
        # into 3 lanes at bases {0,32,64} (~8us of the post-AllGather
        # pack); instead the per-r broadcast selects row r with a
        # host-shipped one-hot stationary: OH[:, r*128:+125] is
        # all-ones in row r, so OH_sliceT @ lgb = row r replicated
        # across all 125 output partitions. Same N=500 matmul cost.
        lgb = small.tile([R, T], BF16, tag="lgb", name="lgb")
        nc.scalar.activation(lgb[:], lgp[:],
                             mybir.ActivationFunctionType.Exp)

        for ci in range(3, NCH):
            emit_a_passes(ci)

        def lg_slice(r):
            return (lgb[:], oh[:, r * 128:r * 128 + SP])

        # ---------- main loop part B: the logits-gated tail ----------
        for ci in range(NCH):
            r0 = ci * CH
            at = ats[ci]
            ot = opool.tile([SP, CW], BF16, tag="o", name="o")
            ssc = spool.tile([SP, CH], F32, tag="ss", name="ss")
            rsc = spool.tile([SP, CH], F32, tag="rs", name="rs")
            for g in range(CH):
                seg = slice(g * T, (g + 1) * T)
                # broadcast L row r across partitions via a one-hot matmul
                rhs, lhs_onehot = lg_slice(r0 + g)
                bt = bppool.tile([SP, 512], F32, tag="bp", name="bp")
                nc.tensor.matmul(bt[:, :T], lhs_onehot, rhs,
                                 start=True, stop=True)
                # q = a * L_bcast with fused row-sum, written straight
                # into the output chunk slice (normalized in place below;
                # all these ops serialize on DVE program order anyway)
                # (tensor_tensor_reduce fails NEFF-side on this stack;
                # scalar_tensor_tensor with op0=bypass is HW-proven;
                # an ACT-engine PSUM->SBUF bcast copy before the stt
                # took the device down UNRECOVERABLE - do not retry).
                nc.vector.scalar_tensor_tensor(
                    ot[:, seg], bt[:, :T], 0.0, at[:, seg],
                    op0=mybir.AluOpType.bypass, op1=mybir.AluOpType.mult,
                    accum_out=ssc[:, g:g + 1])
            nc.vector.reciprocal(rsc[:], ssc[:])
            HH = CH // 2
            for g in range(CH):
                seg = slice(g * T, (g + 1) * T)
                # normalize stays ENTIRELY on DVE. Measured offloads:
                # ACT Copy+scale ~1us/r; Pool tensor_scalar 7.4us/r AND
                # it dragged concurrent DVE tensor_scalar from 340ns to
                # 4.3us (SBUF 2-port contention). Do not offload.
                nc.vector.tensor_scalar_mul(ot[:, seg], ot[:, seg],
                                            rsc[:, g:g + 1])
                if g == HH - 1:
                    # half-chunk store: the final chunk's store was an
                    # exposed ~10us tail; splitting halves it
                    nc.gpsimd.dma_start(
                        out[:, r0:r0 + HH, :],
                        ot[:, :HH * T].rearrange("p (c t) -> p c t", c=HH))
            nc.gpsimd.dma_start(
                out[:, r0 + HH:r0 + CH, :],
                ot[:, HH * T:].rearrange("p (c t) -> p c t", c=HH))


def _get_nc():
    if "nc" not in _CACHE:
        _CACHE["nc"] = _build()
    return _CACHE["nc"]


def prep_in_maps(inputs):
    import ml_dtypes
    f32 = np.float32
    bf16 = ml_dtypes.bfloat16
    state = np.asarray(inputs["state"], f32)[0]          # (500, 2)
    payoff = np.asarray(inputs["payoff"], f32)           # (500, 500)
    noise = np.asarray(inputs["feat_noise"], f32)[0]     # (500, 2)
    xT = np.concatenate([state, payoff, noise], axis=1).T.copy()  # (504, 500)
    gamma = np.asarray(inputs["bn_gamma"], f32)
    beta = np.asarray(inputs["bn_beta"], f32)
    adjT = (np.asarray(inputs["norm_adj"], f32) * gamma[:, None]).T
    dclT = np.asarray(inputs["def_cur_loc"], f32).T
    wr_full = np.asarray(inputs["actgen_w"], f32).reshape(T, H2, T)
    wr_full = wr_full.transpose(1, 0, 2)                 # (16, 500, 500)
    # per-core 2-channel shards, partition-contiguous:
    # wr_pack[c, p, k*T + t] = wr_full[c, k*125 + p, t]
    wr_all = np.ascontiguousarray(
        wr_full.reshape(H2, KT, 125, T).transpose(0, 2, 1, 3)
    ).reshape(H2, 125, KT * T).astype(bf16)
    # packed param planes (see _build)
    adjb = adjT.astype(bf16)    # (500, 500): k-tile rows k*125..
    avb = np.asarray(inputs["actgen_v"], f32).astype(bf16)
    dclb = dclT.astype(bf16)    # (500, 50)
    pbf = np.concatenate(
        [np.concatenate([adjb[k * 125:(k + 1) * 125] for k in range(KT)], axis=1),
         np.concatenate([avb[k * 125:(k + 1) * 125] for k in range(KT)], axis=1),
         np.concatenate([dclb[k * 125:(k + 1) * 125] for k in range(KT)], axis=1)],
        axis=1)                 # (125, 8*500 + 4*50)
    w1f = np.asarray(inputs["gc1_w"], f32)
    pxb = np.concatenate(
        [np.concatenate([xT[k * 126:(k + 1) * 126] for k in range(KT)], axis=1),
         np.concatenate([w1f[k * 126:(k + 1) * 126] for k in range(KT)], axis=1)],
        axis=1).astype(bf16)    # (126, 4*500 + 4*32)
    ohm = np.zeros((R, R * 128), bf16)
    for r in range(R):
        ohm[r, r * 128:r * 128 + SP] = bf16(1.0)
    common = {
        "pbf": np.ascontiguousarray(pbf),
        "pxb": np.ascontiguousarray(pxb),
        "oh": ohm,
    }
    w2f = np.asarray(inputs["gc2_w"], f32)
    b2f = np.asarray(inputs["gc2_b"], f32).reshape(-1)
    b1f = np.asarray(inputs["gc1_b"], f32).reshape(-1)
    u = np.asarray(inputs["gumbel_u"], f32)              # (1000, 50, 500)
    in_maps = []
    for i in range(NCORES):
        m = dict(common)
        # this core owns gc2 output channels (2i, 2i+1); only those 2
        # columns of w2 (and entries of b2) are shipped/computed
        own = [2 * i, 2 * i + 1]
        m["w2"] = np.ascontiguousarray(w2f[:, own])
        m["rows"] = np.concatenate(
            [b1f, b2f[own], gamma, beta]).reshape(1, -1).astype(bf16)
        # [125, 2*KT*T]: per partition, the 2 owned channels' 8KB runs
        # back to back = one 16KB-contiguous DMA descriptor
        m["wr"] = np.ascontiguousarray(
            wr_all[2 * i:2 * i + 2].transpose(1, 0, 2).reshape(125, -1))
        m["u"] = np.ascontiguousarray(u[i * SP:(i + 1) * SP])  # (125, 50, 500)
        in_maps.append(m)
    return in_maps


def run(inputs, trace=False):
    nc = _get_nc()
    in_maps = prep_in_maps(inputs)
    res = bass_utils.run_bass_kernel_spmd(
        nc, in_maps, core_ids=list(range(NCORES)), trace=trace)
    full = np.concatenate(
        [np.asarray(res.results[i]["out"]).astype(np.float32)
         for i in range(NCORES)], axis=0)                # (1000, 50, 500)
    return full, res


def kernel(**inputs):
    full, _ = run(inputs)
    return full


# revision 40
# speedup vs baseline: 1.0552x; 1.0552x over previous
"""Trainium2 Bass kernel for nn_Def_A2C_Sample_Generator.

Computation (see reference):
  x = concat(state, payoff, noise)            (500, 504)
  h1 = lrelu(bn(adj @ (x @ w1) + b1))         (500, 32)
  h2 = lrelu(bn(adj @ (h1 @ w2) + b2))        (500, 16)
  xf = h2.reshape(8000)
  logits = xf @ actgen_w + def_cur_loc @ actgen_v          (50, 500)
  out = softmax(logits[None] + gumbel(u), axis=-1)         (1000, 50, 500)

Sharding: data-parallel over the 1000 samples, 125 per core on 8
cores; actgen_w is channel-sharded 2-of-16 per core (only the owned 2
h2 channels are computed); the 2KB z partials are joined with an ncfw
AllGather ([1,500] -> [8,500]) + a K=8 f32 ones-matmul into the
logits PSUM (AG floor ~5us vs AllReduce ~10us, and the f32 gather+sum
matches AllReduce numerics).

KEY MEASURED FACTS driving this design (8 traced HW runs):
- Each DMA queue sustains only ~120-130GB/s (either a 5-engine SDMA
  allotment at line rate, or descriptor-emission limits); queues run
  concurrently BUT share the 16 SDMA engines at packet granularity,
  so bulk u streams on other queues STARVE the small param
  descriptors (adjT slipped 13us -> 53us; z trigger 78us). Params
  therefore own the sync-ring head, u follows on the same ring.
- The ncfw collective BARRIER ends at runtime-init-skew time (42 to
  103us across runs!) regardless of when collectives are triggered;
  the first collective starts at barrier_end+11us and runs ~15-26us.
  A warmup collective is NET NEGATIVE (it serializes its own ~15us
  ahead of the z collective and saves nothing). The z AllGather
  completes at barrier_end + ~37us; this jitter dominates run-to-run
  variance and nothing in the kernel can hide it (logits gate all
  per-sample DVE work).
- Engine offloads of the normalize measured: ACT Copy+scale ~1us/r
  (3x DVE), Pool tensor_scalar 7.4us/r AND it degrades concurrent
  DVE ops 12x via SBUF port contention. Everything stays on DVE.
- Emission ORDER is scheduling priority: the logits-gated exp/pack
  emitted before the ACT chunk passes stalls ACT ~45us behind the
  (cost-model-invisible) collective latency; emitted after ALL of
  them, the pack waits for ACT to drain (~130us). It goes mid-queue,
  between chunk 2's and chunk 3's passes.

Softmax factorization keeps all gumbel work independent of logits:
  exp(logits + g) with g = -ln(-ln u) equals L * a where
  L = exp(logits) (prologue row) and a = exp(-ln(-ln u)) = -1/ln u.
Main loop, CH=10 r's per chunk in the natural (sample, r, T) layout:
  a       : 3 chunk-wide ACT passes (Ln, Ln(-x) in-place f32, then
            Exp(-x) into a separate bf16 tile so the u tile recycles
            at ACT pace); one table set via the act-table monkeypatch
  L bcast : per-r PE ones-matmul, single bf16 plane into f32 PSUM
  q, S    : DVE scalar_tensor_tensor mult (bf16 out) + fused row-sum
  out     : one DVE reciprocal per chunk + per-r tensor_scalar mult
            into a bf16 chunk tile, one 1.25MB store per chunk
            (host upcasts to f32; rel err ~5e-3, gate is 2e-2)

u MUST stay f32 (a = -1/ln u amplifies input error by 1/(1-u)).
All matmul operands that tolerate bf16 are bf16. Params are packed
into two planes; the z-sum matmul is f32 (exact partial-sum join).

DMA queues: params (GCN-dependency order) then u chunks 0-3 on the
sync HWDGE ring; wr + u chunk 4 + zin/zag/L-bounces + output stores
on the gpsimd SWDGE ring (idle mid-kernel, so the logits pack fires
the moment the AllGather lands); scalar ring unused for bulk (its
issues stall ACT mid-pass, and mid-kernel its queue sits behind ACT's
pass backlog).

Known-bad variants (all HW-measured, do not retry): Pool-engine
tensor_scalar_mul normalize (7.4us/r + 12x DVE degradation); ACT
normalize offload; ACT-engine PSUM->SBUF bcast copies before the stt
(device UNRECOVERABLE); 2-pass Ln+Reciprocal (no act table has both;
nc.scalar Reciprocal banned); warmup collective; u spread across
scalar/gpsimd ahead of params; replicated-z without the collective
(full xf@actgen_w is 64 N=500 PE matmuls = ~40us serial).
"""
import sys

if "/opt/trn_rl_repo" not in sys.path:
    sys.path.insert(0, "/opt/trn_rl_repo")

import numpy as np

import concourse.bacc as bacc
import concourse.bass as bass
import concourse.mybir as mybir
import concourse.tile as tile
from concourse import bass_utils

# The act-table-load pass resolves Exp -> exp_and_others (id 0) and
# Ln -> natural_log (id 5), thrashing a ~2.7us table swap at every
# Ln<->Exp transition in the main loop. natural_log_exp_and_others
# (id 6) holds BOTH. Hide exp/ln from the other sets in the map the
# chooser reads (ids keep indexing the real act_info.json, so the
# loaded tables are unchanged) so every Exp and Ln lands on set 6 and
# one load suffices.
_orig_get_act_tables = bacc.get_activation_tables


def _patched_get_act_tables(arch):
    tabs = dict(_orig_get_act_tables(arch))
    both = {mybir.ActivationFunctionType.Exp, mybir.ActivationFunctionType.Ln}
    for name, fns in tabs.items():
        if name != "natural_log_exp_and_others" and (both & fns):
            tabs[name] = fns - both
    return tabs


bacc.get_activation_tables = _patched_get_act_tables

F32 = mybir.dt.float32
BF16 = mybir.dt.bfloat16
NCORES = 8
T = 500
R = 50
NS = 1000
SP = NS // NCORES  # 125 samples per core
H1, H2 = 32, 16
FIN = 504  # 2 + 500 + 2 input features
KT = 4  # K/M tiling of the 500 dim into 4x125
NEG_SLOPE = 0.2
CH = 10  # r's per chunk: 20KB per-partition DMA descriptors
NCH = R // CH

_CACHE = {}


def _build():
    nc = bacc.Bacc("TRN2", target_bir_lowering=False, debug=False,
                   enable_asserts=False, num_devices=NCORES)

    # ---- I/O ----
    din = {}
    # pbf[p, :] = adjT k-tiles (4x500) | av k-tiles (4x500) | dclT (4x50)
    din["pbf"] = nc.dram_tensor("pbf", [125, 8 * T + 4 * R], BF16,
                                kind="ExternalInput")
    # pxb[p, :] = xT k-tiles (4x500) | w1 k-tiles (4x32), bf16
    din["pxb"] = nc.dram_tensor("pxb", [126, 4 * T + 4 * H1], BF16,
                                kind="ExternalInput")
    # rows[0, :] = b1 (32) | b2sel (2) | grow (500) | brow (500)
    din["rows"] = nc.dram_tensor("rows", [1, H1 + 2 + 2 * T], BF16,
                                 kind="ExternalInput")
    # only the 2 owned output channels of gc2
    din["w2"] = nc.dram_tensor("w2", [H1, 2], F32, kind="ExternalInput")
    # per-core actgen_w shard: 2 of 16 channels, packed [p][c][k*T+t]
    # so one DMA moves 16KB contiguous per partition
    din["wr"] = nc.dram_tensor("wr", [125, 2 * KT * T], BF16,
                               kind="ExternalInput")
    # one-hot broadcast stationaries: oh[k, r*128+s] = (k == r)
    din["oh"] = nc.dram_tensor("oh", [R, R * 128], BF16,
                               kind="ExternalInput")
    din["u"] = nc.dram_tensor("u", [SP, R, T], F32, kind="ExternalInput")
    out = nc.dram_tensor("out", [SP, R, T], BF16, kind="ExternalOutput")

    with tile.TileContext(nc) as tc:
        _emit(nc, tc, din, out)
    nc.compile()
    return nc


def _emit(nc, tc, din, out):
    from contextlib import ExitStack

    ctx = ExitStack()
    with ctx:
        # ---------- pools ----------
        const = ctx.enter_context(tc.tile_pool(name="const", bufs=1))
        small = ctx.enter_context(tc.tile_pool(name="small", bufs=1))
        psum = ctx.enter_context(tc.tile_pool(name="psum", bufs=1, space="PSUM"))
        dram = ctx.enter_context(tc.tile_pool(name="dram", bufs=1, space="DRAM"))

        CW = CH * T
        upool = ctx.enter_context(tc.tile_pool(name="upool", bufs=3))
        apool = ctx.enter_context(tc.tile_pool(name="apool", bufs=NCH))
        opool = ctx.enter_context(tc.tile_pool(name="opool", bufs=3))
        spool = ctx.enter_context(tc.tile_pool(name="spool", bufs=2))
        bppool = ctx.enter_context(tc.tile_pool(name="bppool", bufs=5,
                                                space="PSUM"))

        onesb = const.tile([65, 128], BF16, tag="onesb", name="onesb")
        nc.vector.memset(onesb[:], 1.0)
        ones32 = const.tile([8, R], F32, tag="ones32", name="ones32")
        nc.vector.memset(ones32[:], 1.0)

        # (NO warmup collective: measured across 5 runs, the ncfw
        # barrier ends at runtime-init-skew time (42-103us) regardless
        # of when collectives are triggered, and the first collective
        # starts at barrier_end+11us. A warmup therefore only ADDS its
        # own ~15us execution ahead of the z AllGather.)

        # ---------- param loads, all on the sync ring in GCN-dependency
        # order (measured: the sync HWDGE queue runs ~130GB/s flat -- 5
        # SDMA engines at line rate -- and the scalar queue is
        # emission-limited and SLOW for small descriptors, so params
        # belong on sync; bulk u chunks go to the other queues) --------
        pxb = const.tile([126, 4 * T + 4 * H1], BF16, tag="pxb", name="pxb")
        nc.sync.dma_start(pxb[:], din["pxb"][:])
        PHALF = 4 * T  # adjT k-tiles first (unblocks a1ps); av+dclT only
        # feed the early lgv matmuls, so they load after rows/w2
        pbf = const.tile([125, 8 * T + 4 * R], BF16, tag="pbf", name="pbf")
        nc.sync.dma_start(pbf[:, :PHALF], din["pbf"][:, :PHALF])
        rows = const.tile([1, H1 + 2 + 2 * T], BF16, tag="rows", name="rows")
        nc.sync.dma_start(rows[:], din["rows"][:])
        w2 = const.tile([H1, 2], F32, tag="w2", name="w2")
        nc.sync.dma_start(w2[:], din["w2"][:])
        nc.sync.dma_start(pbf[:, PHALF:], din["pbf"][:, PHALF:])
        oh = const.tile([R, R * 128], BF16, tag="oh", name="oh")
        nc.sync.dma_start(oh[:], din["oh"][:])

        adjT = [pbf[:, k * T:(k + 1) * T] for k in range(KT)]
        av = [pbf[:, (KT + k) * T:(KT + k + 1) * T] for k in range(KT)]
        dclT = [pbf[:, 8 * T + k * R:8 * T + (k + 1) * R] for k in range(KT)]
        w1 = [pxb[:, 4 * T + k * H1:4 * T + (k + 1) * H1] for k in range(KT)]
        b1 = rows[0:1, 0:H1]
        b2 = rows[0:1, H1:H1 + 2]
        GB = H1 + 2  # rows-pack offset of grow
        grow = rows[0:1, GB:GB + T]
        brow = rows[0:1, GB + T:GB + 2 * T]

        # per-core wr shard (2 channels, 1MB, one 16KB-per-partition
        # DMA). MUST be emitted before the u chunks that share the
        # gpsimd ring: SWDGE drains in emission order and wr gates the
        # z partial -> collective -> the whole logits-dependent tail.
        wpool = ctx.enter_context(tc.tile_pool(name="wpool", bufs=1))
        wrm = wpool.tile([125, 2 * KT * T], BF16, tag="wr_stream",
                         name="wr_stream")
        nc.gpsimd.dma_start(wrm[:], din["wr"][:])
        wgs = [wrm[:, c * KT * T:(c + 1) * KT * T] for c in range(2)]

        # u stream: chunks 0-3 on the sync ring BEHIND the params,
        # chunk 4 on gpsimd behind wr. Spreading u wider was tried
        # twice and made things WORSE: SDMA engines round-robin between
        # queues at packet granularity, so concurrent big u packets
        # starve the small param descriptors (adjT landed at 53us
        # instead of 13, pushing the z trigger out). One early c4 on
        # the otherwise-idle gpsimd ring pulls the last a-chunk in by
        # ~25us without meaningfully contending the param window.
        # (c0 gets its own 1-buf pool so its gpsimd DMA never waits on
        # an upool slot release -- a slot wait would head-of-line-block
        # the zin bounce queued behind it. c0 rides gpsimd because ACT
        # consumes it FIRST: it lands ~26us vs ~44us behind the params
        # on sync, pulling the whole ACT pipeline earlier.)
        u2pool = ctx.enter_context(tc.tile_pool(name="u2pool", bufs=1))
        uts = []
        for ci in range(NCH):
            if ci == 0:
                ut = u2pool.tile([SP, CW], F32, tag="u0", name="u0")
                nc.gpsimd.dma_start(
                    ut[:].rearrange("p (c t) -> p c t", c=CH),
                    din["u"][:, ci * CH:(ci + 1) * CH, :])
            else:
                ut = upool.tile([SP, CW], F32, tag="u", name="u")
                nc.sync.dma_start(
                    ut[:].rearrange("p (c t) -> p c t", c=CH),
                    din["u"][:, ci * CH:(ci + 1) * CH, :])
            uts.append(ut)

        # ---------- logits av part, accumulated EARLY (independent of
        # z); the PSUM group stays open until the z-sum matmul ----------
        lgp = psum.tile([R, T], F32, tag="ps_lg", name="ps_lg")
        for k in range(KT):
            nc.tensor.matmul(lgp[:], dclT[k], av[k],
                             start=(k == 0), stop=False)

        # ---------- GCN, transposed formulation ----------
        # bn folded into the adjacency host-side (adjT ships
        # gamma[t]*adj[t,u] transposed), leaving rank-1 bias terms.
        # Only the 2 owned h2 channels are computed (w2 ships 2 cols).
        def lrelu_from_psum(ps_ap, out_tile, width):
            tmp = small.tile([width, T], F32, tag=f"lr{width}", name=f"lr{width}")
            nc.vector.tensor_scalar_mul(tmp[:], ps_ap, NEG_SLOPE)
            nc.vector.tensor_tensor(out_tile[:], tmp[:], ps_ap,
                                    op=mybir.AluOpType.max)

        xw1 = [small.tile([125, H1], BF16, tag=f"xw1{m}", name=f"xw1{m}") for m in range(KT)]
        for m in range(KT):
            ps = psum.tile([125, H1], F32, tag="ps_small", name="ps_small")
            for k in range(KT):
                nc.tensor.matmul(ps[:], pxb[:, k * T + m * 125:k * T + (m + 1) * 125],
                                 w1[k], start=(k == 0), stop=(k == KT - 1))
            nc.vector.tensor_copy(xw1[m][:], ps[:])

        a1ps = psum.tile([H1, T], F32, tag="ps_small", name="ps_small")
        for k in range(KT):
            nc.tensor.matmul(a1ps[:], xw1[k][:], adjT[k],
                             start=(k == 0), stop=False)
        nc.tensor.matmul(a1ps[:], b1, grow, start=False, stop=False)
        nc.tensor.matmul(a1ps[:], onesb[0:1, :H1], brow, start=False,
                         stop=True)
        h1T = small.tile([H1, T], F32, tag="h1T", name="h1T")
        lrelu_from_psum(a1ps[:], h1T, H1)

        xw2 = [small.tile([125, 2], BF16, tag=f"xw2{m}", name=f"xw2{m}") for m in range(KT)]
        for m in range(KT):
            ps = psum.tile([125, 2], F32, tag="ps_small", name="ps_small")
            nc.tensor.matmul(ps[:], h1T[:, m * 125:(m + 1) * 125], w2[:],
                             start=True, stop=True)
            nc.vector.tensor_copy(xw2[m][:], ps[:])

        # layer 2, only the 2 owned channels, in [t, c] layout
        h2b = [small.tile([125, 2], BF16, tag=f"h2b{m}", name=f"h2b{m}")
               for m in range(KT)]
        for m in range(KT):
            m0 = m * 125
            pt = psum.tile([125, 2], F32, tag="ps_small", name="ps_small")
            for k in range(KT):
                nc.tensor.matmul(pt[:], pbf[:, k * T + m0:k * T + m0 + 125],
                                 xw2[k][:], start=(k == 0), stop=False)
            nc.tensor.matmul(pt[:], rows[0:1, GB + m0:GB + m0 + 125], b2,
                             start=False, stop=False)
            nc.tensor.matmul(pt[:], rows[0:1, GB + T + m0:GB + T + m0 + 125],
                             onesb[0:1, :2], start=False, stop=True)
            tmp2 = small.tile([125, 2], F32, tag=f"lr2{m}", name=f"lr2{m}")
            nc.vector.tensor_scalar_mul(tmp2[:], pt[:], NEG_SLOPE)
            nc.vector.tensor_tensor(h2b[m][:], tmp2[:], pt[:],
                                    op=mybir.AluOpType.max)

        # ---------- z partial (2 owned channels) + AllGather ----------
        zps = psum.tile([1, T], F32, tag="ps_z", name="ps_z")
        first = True
        for c in range(2):
            wt = wgs[c]
            for k in range(KT):
                nc.tensor.matmul(zps[:], h2b[k][:, c:c + 1],
                                 wt[:, k * T:(k + 1) * T],
                                 start=first,
                                 stop=(c == 1 and k == KT - 1))
                first = False
        zpart = small.tile([1, T], F32, tag="zpart", name="zpart")
        nc.vector.tensor_copy(zpart[:], zps[:])
        zin = dram.tile([1, T], F32, name="zin")
        zag_d = dram.tile([NCORES, T], F32, addr_space="Shared", name="zag_d")
        # bounce DMAs on the gpsimd ring: sync is mid-u-stream (HOL
        # block) and a scalar-ring issue would stall ACT mid-pass
        nc.gpsimd.dma_start(zin[:], zpart[:])
        nc.gpsimd.collective_compute(
            "AllGather", mybir.AluOpType.bypass,
            replica_groups=[list(range(NCORES))],
            ins=[zin.opt()], outs=[zag_d.opt()])
        # ---------- main loop part A: the gumbel a = -1/ln(u) passes.
        # The logits-gated exp/pack is emitted BETWEEN chunk 1's and
        # chunk 2's passes (ACT interleaves the remaining passes with
        # DVE's early chunks even when the AllGather lands late): emission order is the scheduler's static
        # priority, so exp-first stalls ACT ~45us behind the
        # (cost-model-invisible) AllGather, while exp-last delays the
        # pack until ACT drains all passes (~130us). Mid-queue splits
        # the difference for the observed AllGather-landing range.
        ats = []

        def emit_a_passes(ci):
            ut = uts[ci]
            nc.scalar.activation(ut[:], ut[:], mybir.ActivationFunctionType.Ln)
            nc.scalar.activation(ut[:], ut[:], mybir.ActivationFunctionType.Ln,
                                 scale=-1.0)
            at = apool.tile([SP, CW], BF16, tag="a", name="a")
            nc.scalar.activation(at[:], ut[:], mybir.ActivationFunctionType.Exp,
                                 scale=-1.0)
            ats.append(at)

        for ci in range(2):
            emit_a_passes(ci)

        zag = small.tile([NCORES, T], F32, tag="zag", name="zag")
        nc.gpsimd.dma_start(zag[:], zag_d[:])

        # z-sum joined into the logits PSUM with a K=8 f32 ones-matmul
        nc.tensor.matmul(lgp[:], ones32[:, :R], zag[:], start=False, stop=True)

        # L = exp(logits) stays in its natural [50, 500] layout (base
        # partition 0, legal). The old design bounced rows through DRAM
        # into 3 lanes at bases {0,32,64} (~8us of the post-AllGather
        # pack); instead the per-r broadcast selects row r with a
        # host-shipped one-hot stationary: OH[:, r*128:+125] is
        # all-ones in row r, so OH_sliceT @ lgb = row r replicated
        # across all 125 output partitions. Same N=500 matmul cost.
        lgb = small.tile([R, T], BF16, tag="lgb", name="lgb")
        nc.scalar.activation(lgb[:], lgp[:],
                             mybir.ActivationFunctionType.Exp)

        for ci in range(2, NCH):
            emit_a_passes(ci)

        def lg_slice(r):
            return (lgb[:], oh[:, r * 128:r * 128 + SP])

        # ---------- main loop part B: the logits-gated tail ----------
        for ci in range(NCH):
            r0 = ci * CH
            at = ats[ci]
            ot = opool.tile([SP, CW], BF16, tag="o", name="o")
            ssc = spool.tile([SP, CH], F32, tag="ss", name="ss")
            rsc = spool.tile([SP, CH], F32, tag="rs", name="rs")
            for g in range(CH):
                seg = slice(g * T, (g + 1) * T)
                # broadcast L row r across partitions via a one-hot matmul
                rhs, lhs_onehot = lg_slice(r0 + g)
                bt = bppool.tile([SP, 512], F32, tag="bp", name="bp")
                nc.tensor.matmul(bt[:, :T], lhs_onehot, rhs,
                                 start=True, stop=True)
                # q = a * L_bcast with fused row-sum, written straight
                # into the output chunk slice (normalized in place below;
                # all these ops serialize on DVE program order anyway)
                # (tensor_tensor_reduce fails NEFF-side on this stack;
                # scalar_tensor_tensor with op0=bypass is HW-proven;
                # an ACT-engine PSUM->SBUF bcast copy before the stt
                # took the device down UNRECOVERABLE - do not retry).
                nc.vector.scalar_tensor_tensor(
                    ot[:, seg], bt[:, :T], 0.0, at[:, seg],
                    op0=mybir.AluOpType.bypass, op1=mybir.AluOpType.mult,
                    accum_out=ssc[:, g:g + 1])
            nc.vector.reciprocal(rsc[:], ssc[:])
            HH = CH // 2
            for g in range(CH):
                seg = slice(g * T, (g + 1) * T)
                # normalize stays ENTIRELY on DVE. Measured offloads:
                # ACT Copy+scale ~1us/r; Pool tensor_scalar 7.4us/r AND
                # it dragged concurrent DVE tensor_scalar from 340ns to
                # 4.3us (SBUF 2-port contention). Do not offload.
                nc.vector.tensor_scalar_mul(ot[:, seg], ot[:, seg],
                                            rsc[:, g:g + 1])
                if g == HH - 1:
                    # half-chunk store: the final chunk's store was an
                    # exposed ~10us tail; splitting halves it
                    nc.gpsimd.dma_start(
                        out[:, r0:r0 + HH, :],
                        ot[:, :HH * T].rearrange("p (c t) -> p c t", c=HH))
            # last chunk: second half-store rides the (idle by now) sync
            # ring so the two final half-stores drain in parallel
            seng = nc.sync if ci == NCH - 1 else nc.gpsimd
            seng.dma_start(
                out[:, r0 + HH:r0 + CH, :],
                ot[:, HH * T:].rearrange("p (c t) -> p c t", c=HH))


def _get_nc():
    if "nc" not in _CACHE:
        _CACHE["nc"] = _build()
    return _CACHE["nc"]


def prep_in_maps(inputs):
    import ml_dtypes
    f32 = np.float32
    bf16 = ml_dtypes.bfloat16
    state = np.asarray(inputs["state"], f32)[0]          # (500, 2)
    payoff = np.asarray(inputs["payoff"], f32)           # (500, 500)
    noise = np.asarray(inputs["feat_noise"], f32)[0]     # (500, 2)
    xT = np.concatenate([state, payoff, noise], axis=1).T.copy()  # (504, 500)
    gamma = np.asarray(inputs["bn_gamma"], f32)
    beta = np.asarray(inputs["bn_beta"], f32)
    adjT = (np.asarray(inputs["norm_adj"], f32) * gamma[:, None]).T
    dclT = np.asarray(inputs["def_cur_loc"], f32).T
    wr_full = np.asarray(inputs["actgen_w"], f32).reshape(T, H2, T)
    wr_full = wr_full.transpose(1, 0, 2)                 # (16, 500, 500)
    # per-core 2-channel shards, partition-contiguous:
    # wr_pack[c, p, k*T + t] = wr_full[c, k*125 + p, t]
    wr_all = np.ascontiguousarray(
        wr_full.reshape(H2, KT, 125, T).transpose(0, 2, 1, 3)
    ).reshape(H2, 125, KT * T).astype(bf16)
    # packed param planes (see _build)
    adjb = adjT.astype(bf16)    # (500, 500): k-tile rows k*125..
    avb = np.asarray(inputs["actgen_v"], f32).astype(bf16)
    dclb = dclT.astype(bf16)    # (500, 50)
    pbf = np.concatenate(
        [np.concatenate([adjb[k * 125:(k + 1) * 125] for k in range(KT)], axis=1),
         np.concatenate([avb[k * 125:(k + 1) * 125] for k in range(KT)], axis=1),
         np.concatenate([dclb[k * 125:(k + 1) * 125] for k in range(KT)], axis=1)],
        axis=1)                 # (125, 8*500 + 4*50)
    w1f = np.asarray(inputs["gc1_w"], f32)
    pxb = np.concatenate(
        [np.concatenate([xT[k * 126:(k + 1) * 126] for k in range(KT)], axis=1),
         np.concatenate([w1f[k * 126:(k + 1) * 126] for k in range(KT)], axis=1)],
        axis=1).astype(bf16)    # (126, 4*500 + 4*32)
    ohm = np.zeros((R, R * 128), bf16)
    for r in range(R):
        ohm[r, r * 128:r * 128 + SP] = bf16(1.0)
    common = {
        "pbf": np.ascontiguousarray(pbf),
        "pxb": np.ascontiguousarray(pxb),
        "oh": ohm,
    }
    w2f = np.asarray(inputs["gc2_w"], f32)
    b2f = np.asarray(inputs["gc2_b"], f32).reshape(-1)
    b1f = np.asarray(inputs["gc1_b"], f32).reshape(-1)
    u = np.asarray(inputs["gumbel_u"], f32)              # (1000, 50, 500)
    in_maps = []
    for i in range(NCORES):
        m = dict(common)
        # this core owns gc2 output channels (2i, 2i+1); only those 2
        # columns of w2 (and entries of b2) are shipped/computed
        own = [2 * i, 2 * i + 1]
        m["w2"] = np.ascontiguousarray(w2f[:, own])
        m["rows"] = np.concatenate(
            [b1f, b2f[own], gamma, beta]).reshape(1, -1).astype(bf16)
        # [125, 2*KT*T]: per partition, the 2 owned channels' 8KB runs
        # back to back = one 16KB-contiguous DMA descriptor
        m["wr"] = np.ascontiguousarray(
            wr_all[2 * i:2 * i + 2].transpose(1, 0, 2).reshape(125, -1))
        m["u"] = np.ascontiguousarray(u[i * SP:(i + 1) * SP])  # (125, 50, 500)
        in_maps.append(m)
    return in_maps


def run(inputs, trace=False):
    nc = _get_nc()
    in_maps = prep_in_maps(inputs)
    res = bass_utils.run_bass_kernel_spmd(
        nc, in_maps, core_ids=list(range(NCORES)), trace=trace)
    full = np.concatenate(
        [np.asarray(res.results[i]["out"]).astype(np.float32)
         for i in range(NCORES)], axis=0)                # (1000, 50, 500)
    return full, res


def kernel(**inputs):
    full, _ = run(inputs)
    return full


# revision 41
# speedup vs baseline: 1.1000x; 1.0424x over previous
"""Trainium2 Bass kernel for nn_Def_A2C_Sample_Generator.

Computation (see reference):
  x = concat(state, payoff, noise)            (500, 504)
  h1 = lrelu(bn(adj @ (x @ w1) + b1))         (500, 32)
  h2 = lrelu(bn(adj @ (h1 @ w2) + b2))        (500, 16)
  xf = h2.reshape(8000)
  logits = xf @ actgen_w + def_cur_loc @ actgen_v          (50, 500)
  out = softmax(logits[None] + gumbel(u), axis=-1)         (1000, 50, 500)

Sharding: data-parallel over the 1000 samples, 125 per core on 8
cores; actgen_w is channel-sharded 2-of-16 per core (only the owned 2
h2 channels are computed); the 2KB z partials are joined with an ncfw
AllGather ([1,500] -> [8,500]) + a K=8 f32 ones-matmul into the
logits PSUM (AG floor ~5us vs AllReduce ~10us, and the f32 gather+sum
matches AllReduce numerics).

KEY MEASURED FACTS driving this design (8 traced HW runs):
- Each DMA queue sustains only ~120-130GB/s (either a 5-engine SDMA
  allotment at line rate, or descriptor-emission limits); queues run
  concurrently BUT share the 16 SDMA engines at packet granularity,
  so bulk u streams on other queues STARVE the small param
  descriptors (adjT slipped 13us -> 53us; z trigger 78us). Params
  therefore own the sync-ring head, u follows on the same ring.
- The ncfw collective BARRIER ends at runtime-init-skew time (42 to
  103us across runs!) regardless of when collectives are triggered;
  the first collective starts at barrier_end+11us and runs ~15-26us.
  A warmup collective is NET NEGATIVE (it serializes its own ~15us
  ahead of the z collective and saves nothing). The z AllGather
  completes at barrier_end + ~37us; this jitter dominates run-to-run
  variance and nothing in the kernel can hide it (logits gate all
  per-sample DVE work).
- Engine offloads of the normalize measured: ACT Copy+scale ~1us/r
  (3x DVE), Pool tensor_scalar 7.4us/r AND it degrades concurrent
  DVE ops 12x via SBUF port contention. Everything stays on DVE.
- Emission ORDER is scheduling priority: the logits-gated exp/pack
  emitted before the ACT chunk passes stalls ACT ~45us behind the
  (cost-model-invisible) collective latency; emitted after ALL of
  them, the pack waits for ACT to drain (~130us). It goes mid-queue,
  between chunk 2's and chunk 3's passes.

Softmax factorization keeps all gumbel work independent of logits:
  exp(logits + g) with g = -ln(-ln u) equals L * a where
  L = exp(logits) (prologue row) and a = exp(-ln(-ln u)) = -1/ln u.
Main loop, CH=10 r's per chunk in the natural (sample, r, T) layout:
  a       : 3 chunk-wide ACT passes (Ln, Ln(-x) in-place f32, then
            Exp(-x) into a separate bf16 tile so the u tile recycles
            at ACT pace); one table set via the act-table monkeypatch
  L bcast : per-r PE ones-matmul, single bf16 plane into f32 PSUM
  q, S    : DVE scalar_tensor_tensor mult (bf16 out) + fused row-sum
  out     : one DVE reciprocal per chunk + per-r tensor_scalar mult
            into a bf16 chunk tile, one 1.25MB store per chunk
            (host upcasts to f32; rel err ~5e-3, gate is 2e-2)

u MUST stay f32 (a = -1/ln u amplifies input error by 1/(1-u)).
All matmul operands that tolerate bf16 are bf16. Params are packed
into two planes; the z-sum matmul is f32 (exact partial-sum join).

DMA queues: params (GCN-dependency order) then u chunks 0-3 on the
sync HWDGE ring; wr + u chunk 4 + zin/zag/L-bounces + output stores
on the gpsimd SWDGE ring (idle mid-kernel, so the logits pack fires
the moment the AllGather lands); scalar ring unused for bulk (its
issues stall ACT mid-pass, and mid-kernel its queue sits behind ACT's
pass backlog).

Known-bad variants (all HW-measured, do not retry): Pool-engine
tensor_scalar_mul normalize (7.4us/r + 12x DVE degradation); ACT
normalize offload; ACT-engine PSUM->SBUF bcast copies before the stt
(device UNRECOVERABLE); 2-pass Ln+Reciprocal (no act table has both;
nc.scalar Reciprocal banned); warmup collective; u spread across
scalar/gpsimd ahead of params; replicated-z without the collective
(full xf@actgen_w is 64 N=500 PE matmuls = ~40us serial).
"""
import sys

if "/opt/trn_rl_repo" not in sys.path:
    sys.path.insert(0, "/opt/trn_rl_repo")

import numpy as np

import concourse.bacc as bacc
import concourse.bass as bass
import concourse.mybir as mybir
import concourse.tile as tile
from concourse import bass_utils

# The act-table-load pass resolves Exp -> exp_and_others (id 0) and
# Ln -> natural_log (id 5), thrashing a ~2.7us table swap at every
# Ln<->Exp transition in the main loop. natural_log_exp_and_others
# (id 6) holds BOTH. Hide exp/ln from the other sets in the map the
# chooser reads (ids keep indexing the real act_info.json, so the
# loaded tables are unchanged) so every Exp and Ln lands on set 6 and
# one load suffices.
_orig_get_act_tables = bacc.get_activation_tables


def _patched_get_act_tables(arch):
    tabs = dict(_orig_get_act_tables(arch))
    both = {mybir.ActivationFunctionType.Exp, mybir.ActivationFunctionType.Ln}
    for name, fns in tabs.items():
        if name != "natural_log_exp_and_others" and (both & fns):
            tabs[name] = fns - both
    return tabs


bacc.get_activation_tables = _patched_get_act_tables

F32 = mybir.dt.float32
BF16 = mybir.dt.bfloat16
NCORES = 8
T = 500
R = 50
NS = 1000
SP = NS // NCORES  # 125 samples per core
H1, H2 = 32, 16
FIN = 504  # 2 + 500 + 2 input features
KT = 4  # K/M tiling of the 500 dim into 4x125
NEG_SLOPE = 0.2
CH = 10  # r's per chunk: 20KB per-partition DMA descriptors
NCH = R // CH

_CACHE = {}


def _build():
    nc = bacc.Bacc("TRN2", target_bir_lowering=False, debug=False,
                   enable_asserts=False, num_devices=NCORES)

    # ---- I/O ----
    din = {}
    # pbf[p, :] = adjT k-tiles (4x500) | av k-tiles (4x500) | dclT (4x50)
    din["pbf"] = nc.dram_tensor("pbf", [125, 8 * T + 4 * R], BF16,
                                kind="ExternalInput")
    # pxb[p, :] = xT k-tiles (4x500) | w1 k-tiles (4x32), bf16
    din["pxb"] = nc.dram_tensor("pxb", [126, 4 * T + 4 * H1], BF16,
                                kind="ExternalInput")
    # rows[0, :] = b1 (32) | b2sel (2) | grow (500) | brow (500)
    din["rows"] = nc.dram_tensor("rows", [1, H1 + 2 + 2 * T], BF16,
                                 kind="ExternalInput")
    # only the 2 owned output channels of gc2
    din["w2"] = nc.dram_tensor("w2", [H1, 2], F32, kind="ExternalInput")
    # per-core actgen_w shard: 2 of 16 channels, packed [p][c][k*T+t]
    # so one DMA moves 16KB contiguous per partition
    din["wr"] = nc.dram_tensor("wr", [125, 2 * KT * T], BF16,
                               kind="ExternalInput")
    # one-hot broadcast stationaries: oh[k, r*128+s] = (k == r)
    din["oh"] = nc.dram_tensor("oh", [R, R * 128], BF16,
                               kind="ExternalInput")
    din["u"] = nc.dram_tensor("u", [SP, R, T], F32, kind="ExternalInput")
    out = nc.dram_tensor("out", [SP, R, T], BF16, kind="ExternalOutput")

    with tile.TileContext(nc) as tc:
        _emit(nc, tc, din, out)
    nc.compile()
    return nc


def _emit(nc, tc, din, out):
    from contextlib import ExitStack

    ctx = ExitStack()
    with ctx:
        # ---------- pools ----------
        const = ctx.enter_context(tc.tile_pool(name="const", bufs=1))
        small = ctx.enter_context(tc.tile_pool(name="small", bufs=1))
        psum = ctx.enter_context(tc.tile_pool(name="psum", bufs=1, space="PSUM"))
        dram = ctx.enter_context(tc.tile_pool(name="dram", bufs=1, space="DRAM"))

        CW = CH * T
        upool = ctx.enter_context(tc.tile_pool(name="upool", bufs=3))
        apool = ctx.enter_context(tc.tile_pool(name="apool", bufs=NCH))
        opool = ctx.enter_context(tc.tile_pool(name="opool", bufs=3))
        spool = ctx.enter_context(tc.tile_pool(name="spool", bufs=2))
        bppool = ctx.enter_context(tc.tile_pool(name="bppool", bufs=5,
                                                space="PSUM"))

        onesb = const.tile([65, 128], BF16, tag="onesb", name="onesb")
        nc.vector.memset(onesb[:], 1.0)
        ones32 = const.tile([8, R], F32, tag="ones32", name="ones32")
        nc.vector.memset(ones32[:], 1.0)

        # (NO warmup collective: measured across 5 runs, the ncfw
        # barrier ends at runtime-init-skew time (42-103us) regardless
        # of when collectives are triggered, and the first collective
        # starts at barrier_end+11us. A warmup therefore only ADDS its
        # own ~15us execution ahead of the z AllGather.)

        # ---------- param loads, all on the sync ring in GCN-dependency
        # order (measured: the sync HWDGE queue runs ~130GB/s flat -- 5
        # SDMA engines at line rate -- and the scalar queue is
        # emission-limited and SLOW for small descriptors, so params
        # belong on sync; bulk u chunks go to the other queues) --------
        pxb = const.tile([126, 4 * T + 4 * H1], BF16, tag="pxb", name="pxb")
        nc.sync.dma_start(pxb[:], din["pxb"][:])
        PHALF = 4 * T  # adjT k-tiles first (unblocks a1ps); av+dclT only
        # feed the early lgv matmuls, so they load after rows/w2
        pbf = const.tile([125, 8 * T + 4 * R], BF16, tag="pbf", name="pbf")
        nc.sync.dma_start(pbf[:, :PHALF], din["pbf"][:, :PHALF])
        rows = const.tile([1, H1 + 2 + 2 * T], BF16, tag="rows", name="rows")
        nc.sync.dma_start(rows[:], din["rows"][:])
        w2 = const.tile([H1, 2], F32, tag="w2", name="w2")
        nc.sync.dma_start(w2[:], din["w2"][:])
        nc.sync.dma_start(pbf[:, PHALF:], din["pbf"][:, PHALF:])
        oh = const.tile([R, R * 128], BF16, tag="oh", name="oh")
        nc.sync.dma_start(oh[:], din["oh"][:])

        adjT = [pbf[:, k * T:(k + 1) * T] for k in range(KT)]
        av = [pbf[:, (KT + k) * T:(KT + k + 1) * T] for k in range(KT)]
        dclT = [pbf[:, 8 * T + k * R:8 * T + (k + 1) * R] for k in range(KT)]
        w1 = [pxb[:, 4 * T + k * H1:4 * T + (k + 1) * H1] for k in range(KT)]
        b1 = rows[0:1, 0:H1]
        b2 = rows[0:1, H1:H1 + 2]
        GB = H1 + 2  # rows-pack offset of grow
        grow = rows[0:1, GB:GB + T]
        brow = rows[0:1, GB + T:GB + 2 * T]

        # per-core wr shard (2 channels, 1MB, one 16KB-per-partition
        # DMA). MUST be emitted before the u chunks that share the
        # gpsimd ring: SWDGE drains in emission order and wr gates the
        # z partial -> collective -> the whole logits-dependent tail.
        wpool = ctx.enter_context(tc.tile_pool(name="wpool", bufs=1))
        wrm = wpool.tile([125, 2 * KT * T], BF16, tag="wr_stream",
                         name="wr_stream")
        nc.gpsimd.dma_start(wrm[:], din["wr"][:])
        wgs = [wrm[:, c * KT * T:(c + 1) * KT * T] for c in range(2)]

        # u stream: chunks 0-3 on the sync ring BEHIND the params,
        # chunk 4 on gpsimd behind wr. Spreading u wider was tried
        # twice and made things WORSE: SDMA engines round-robin between
        # queues at packet granularity, so concurrent big u packets
        # starve the small param descriptors (adjT landed at 53us
        # instead of 13, pushing the z trigger out). One early c4 on
        # the otherwise-idle gpsimd ring pulls the last a-chunk in by
        # ~25us without meaningfully contending the param window.
        # (c0 gets its own 1-buf pool so its gpsimd DMA never waits on
        # an upool slot release -- a slot wait would head-of-line-block
        # the zin bounce queued behind it. c0 rides gpsimd because ACT
        # consumes it FIRST: it lands ~26us vs ~44us behind the params
        # on sync, pulling the whole ACT pipeline earlier.)
        u2pool = ctx.enter_context(tc.tile_pool(name="u2pool", bufs=1))
        uts = []
        for ci in range(NCH):
            if ci == 0:
                ut = u2pool.tile([SP, CW], F32, tag="u0", name="u0")
                nc.gpsimd.dma_start(
                    ut[:].rearrange("p (c t) -> p c t", c=CH),
                    din["u"][:, ci * CH:(ci + 1) * CH, :])
            else:
                ut = upool.tile([SP, CW], F32, tag="u", name="u")
                nc.sync.dma_start(
                    ut[:].rearrange("p (c t) -> p c t", c=CH),
                    din["u"][:, ci * CH:(ci + 1) * CH, :])
            uts.append(ut)

        # ---------- logits av part, accumulated EARLY (independent of
        # z); the PSUM group stays open until the z-sum matmul ----------
        lgp = psum.tile([R, T], F32, tag="ps_lg", name="ps_lg")
        for k in range(KT):
            nc.tensor.matmul(lgp[:], dclT[k], av[k],
                             start=(k == 0), stop=False)

        # ---------- GCN, transposed formulation ----------
        # bn folded into the adjacency host-side (adjT ships
        # gamma[t]*adj[t,u] transposed), leaving rank-1 bias terms.
        # Only the 2 owned h2 channels are computed (w2 ships 2 cols).
        def lrelu_from_psum(ps_ap, out_tile, width):
            tmp = small.tile([width, T], F32, tag=f"lr{width}", name=f"lr{width}")
            nc.vector.tensor_scalar_mul(tmp[:], ps_ap, NEG_SLOPE)
            nc.vector.tensor_tensor(out_tile[:], tmp[:], ps_ap,
                                    op=mybir.AluOpType.max)

        xw1 = [small.tile([125, H1], BF16, tag=f"xw1{m}", name=f"xw1{m}") for m in range(KT)]
        for m in range(KT):
            ps = psum.tile([125, H1], F32, tag="ps_small", name="ps_small")
            for k in range(KT):
                nc.tensor.matmul(ps[:], pxb[:, k * T + m * 125:k * T + (m + 1) * 125],
                                 w1[k], start=(k == 0), stop=(k == KT - 1))
            nc.vector.tensor_copy(xw1[m][:], ps[:])

        a1ps = psum.tile([H1, T], F32, tag="ps_small", name="ps_small")
        for k in range(KT):
            nc.tensor.matmul(a1ps[:], xw1[k][:], adjT[k],
                             start=(k == 0), stop=False)
        nc.tensor.matmul(a1ps[:], b1, grow, start=False, stop=False)
        nc.tensor.matmul(a1ps[:], onesb[0:1, :H1], brow, start=False,
                         stop=True)
        h1T = small.tile([H1, T], F32, tag="h1T", name="h1T")
        lrelu_from_psum(a1ps[:], h1T, H1)

        xw2 = [small.tile([125, 2], BF16, tag=f"xw2{m}", name=f"xw2{m}") for m in range(KT)]
        for m in range(KT):
            ps = psum.tile([125, 2], F32, tag="ps_small", name="ps_small")
            nc.tensor.matmul(ps[:], h1T[:, m * 125:(m + 1) * 125], w2[:],
                             start=True, stop=True)
            nc.vector.tensor_copy(xw2[m][:], ps[:])

        # layer 2, only the 2 owned channels, in [t, c] layout
        h2b = [small.tile([125, 2], BF16, tag=f"h2b{m}", name=f"h2b{m}")
               for m in range(KT)]
        for m in range(KT):
            m0 = m * 125
            pt = psum.tile([125, 2], F32, tag="ps_small", name="ps_small")
            for k in range(KT):
                nc.tensor.matmul(pt[:], pbf[:, k * T + m0:k * T + m0 + 125],
                                 xw2[k][:], start=(k == 0), stop=False)
            nc.tensor.matmul(pt[:], rows[0:1, GB + m0:GB + m0 + 125], b2,
                             start=False, stop=False)
            nc.tensor.matmul(pt[:], rows[0:1, GB + T + m0:GB + T + m0 + 125],
                             onesb[0:1, :2], start=False, stop=True)
            tmp2 = small.tile([125, 2], F32, tag=f"lr2{m}", name=f"lr2{m}")
            nc.vector.tensor_scalar_mul(tmp2[:], pt[:], NEG_SLOPE)
            nc.vector.tensor_tensor(h2b[m][:], tmp2[:], pt[:],
                                    op=mybir.AluOpType.max)

        # ---------- z partial (2 owned channels) + AllGather ----------
        zps = psum.tile([1, T], F32, tag="ps_z", name="ps_z")
        first = True
        for c in range(2):
            wt = wgs[c]
            for k in range(KT):
                nc.tensor.matmul(zps[:], h2b[k][:, c:c + 1],
                                 wt[:, k * T:(k + 1) * T],
                                 start=first,
                                 stop=(c == 1 and k == KT - 1))
                first = False
        zpart = small.tile([1, T], F32, tag="zpart", name="zpart")
        nc.vector.tensor_copy(zpart[:], zps[:])
        zin = dram.tile([1, T], F32, name="zin")
        zag_d = dram.tile([NCORES, T], F32, addr_space="Shared", name="zag_d")
        # bounce DMAs on the gpsimd ring: sync is mid-u-stream (HOL
        # block) and a scalar-ring issue would stall ACT mid-pass
        nc.gpsimd.dma_start(zin[:], zpart[:])
        nc.gpsimd.collective_compute(
            "AllGather", mybir.AluOpType.bypass,
            replica_groups=[list(range(NCORES))],
            ins=[zin.opt()], outs=[zag_d.opt()])
        # ---------- main loop part A: the gumbel a = -1/ln(u) passes.
        # The logits-gated exp/pack is emitted BETWEEN chunk 1's and
        # chunk 2's passes (ACT interleaves the remaining passes with
        # DVE's early chunks even when the AllGather lands late): emission order is the scheduler's static
        # priority, so exp-first stalls ACT ~45us behind the
        # (cost-model-invisible) AllGather, while exp-last delays the
        # pack until ACT drains all passes (~130us). Mid-queue splits
        # the difference for the observed AllGather-landing range.
        ats = []

        def emit_a_passes(ci):
            ut = uts[ci]
            nc.scalar.activation(ut[:], ut[:], mybir.ActivationFunctionType.Ln)
            nc.scalar.activation(ut[:], ut[:], mybir.ActivationFunctionType.Ln,
                                 scale=-1.0)
            at = apool.tile([SP, CW], BF16, tag="a", name="a")
            nc.scalar.activation(at[:], ut[:], mybir.ActivationFunctionType.Exp,
                                 scale=-1.0)
            ats.append(at)

        for ci in range(2):
            emit_a_passes(ci)

        zag = small.tile([NCORES, T], F32, tag="zag", name="zag")
        nc.gpsimd.dma_start(zag[:], zag_d[:])

        # z-sum joined into the logits PSUM with a K=8 f32 ones-matmul
        nc.tensor.matmul(lgp[:], ones32[:, :R], zag[:], start=False, stop=True)

        # L = exp(logits) stays in its natural [50, 500] layout (base
        # partition 0, legal). The old design bounced rows through DRAM
        # into 3 lanes at bases {0,32,64} (~8us of the post-AllGather
        # pack); instead the per-r broadcast selects row r with a
        # host-shipped one-hot stationary: OH[:, r*128:+125] is
        # all-ones in row r, so OH_sliceT @ lgb = row r replicated
        # across all 125 output partitions. Same N=500 matmul cost.
        lgb = small.tile([R, T], BF16, tag="lgb", name="lgb")
        nc.scalar.activation(lgb[:], lgp[:],
                             mybir.ActivationFunctionType.Exp)

        for ci in range(2, NCH):
            emit_a_passes(ci)

        def lg_slice(r):
            return (lgb[:], oh[:, r * 128:r * 128 + SP])

        # ---------- main loop part B: the logits-gated tail ----------
        for ci in range(NCH):
            r0 = ci * CH
            at = ats[ci]
            ot = opool.tile([SP, CW], BF16, tag="o", name="o")
            ssc = spool.tile([SP, CH], F32, tag="ss", name="ss")
            rsc = spool.tile([SP, CH], F32, tag="rs", name="rs")
            for g in range(CH):
                seg = slice(g * T, (g + 1) * T)
                # broadcast L row r across partitions via a one-hot matmul
                rhs, lhs_onehot = lg_slice(r0 + g)
                bt = bppool.tile([SP, 512], F32, tag="bp", name="bp")
                nc.tensor.matmul(bt[:, :T], lhs_onehot, rhs,
                                 start=True, stop=True)
                # q = a * L_bcast with fused row-sum, written straight
                # into the output chunk slice (normalized in place below;
                # all these ops serialize on DVE program order anyway)
                # (tensor_tensor_reduce fails NEFF-side on this stack;
                # scalar_tensor_tensor with op0=bypass is HW-proven;
                # an ACT-engine PSUM->SBUF bcast copy before the stt
                # took the device down UNRECOVERABLE - do not retry).
                nc.vector.scalar_tensor_tensor(
                    ot[:, seg], bt[:, :T], 0.0, at[:, seg],
                    op0=mybir.AluOpType.bypass, op1=mybir.AluOpType.mult,
                    accum_out=ssc[:, g:g + 1])
            nc.vector.reciprocal(rsc[:], ssc[:])
            HH = CH // 2
            for g in range(CH):
                seg = slice(g * T, (g + 1) * T)
                # normalize stays ENTIRELY on DVE. Measured offloads:
                # ACT Copy+scale ~1us/r; Pool tensor_scalar 7.4us/r AND
                # it dragged concurrent DVE tensor_scalar from 340ns to
                # 4.3us (SBUF 2-port contention). Do not offload.
                nc.vector.tensor_scalar_mul(ot[:, seg], ot[:, seg],
                                            rsc[:, g:g + 1])
                if g == HH - 1:
                    # half-chunk store: the final chunk's store was an
                    # exposed ~10us tail; splitting halves it
                    nc.gpsimd.dma_start(
                        out[:, r0:r0 + HH, :],
                        ot[:, :HH * T].rearrange("p (c t) -> p c t", c=HH))
            if ci == NCH - 1:
                # last chunk: the final store is DMA-descriptor-emission
                # bound (~75ns x 125 partition descriptors ~= 9.4us
                # regardless of bytes) and fully exposed after the last
                # normalize. Split it by partition range across BOTH
                # rings so each emits only ~63 descriptors (~4.7us).
                nc.gpsimd.dma_start(
                    out[0:63, r0 + HH:r0 + CH, :],
                    ot[0:63, HH * T:].rearrange("p (c t) -> p c t", c=HH))
                nc.sync.dma_start(
                    out[63:SP, r0 + HH:r0 + CH, :],
                    ot[63:SP, HH * T:].rearrange("p (c t) -> p c t", c=HH))
            else:
                nc.gpsimd.dma_start(
                    out[:, r0 + HH:r0 + CH, :],
                    ot[:, HH * T:].rearrange("p (c t) -> p c t", c=HH))


def _get_nc():
    if "nc" not in _CACHE:
        _CACHE["nc"] = _build()
    return _CACHE["nc"]


def prep_in_maps(inputs):
    import ml_dtypes
    f32 = np.float32
    bf16 = ml_dtypes.bfloat16
    state = np.asarray(inputs["state"], f32)[0]          # (500, 2)
    payoff = np.asarray(inputs["payoff"], f32)           # (500, 500)
    noise = np.asarray(inputs["feat_noise"], f32)[0]     # (500, 2)
    xT = np.concatenate([state, payoff, noise], axis=1).T.copy()  # (504, 500)
    gamma = np.asarray(inputs["bn_gamma"], f32)
    beta = np.asarray(inputs["bn_beta"], f32)
    adjT = (np.asarray(inputs["norm_adj"], f32) * gamma[:, None]).T
    dclT = np.asarray(inputs["def_cur_loc"], f32).T
    wr_full = np.asarray(inputs["actgen_w"], f32).reshape(T, H2, T)
    wr_full = wr_full.transpose(1, 0, 2)                 # (16, 500, 500)
    # per-core 2-channel shards, partition-contiguous:
    # wr_pack[c, p, k*T + t] = wr_full[c, k*125 + p, t]
    wr_all = np.ascontiguousarray(
        wr_full.reshape(H2, KT, 125, T).transpose(0, 2, 1, 3)
    ).reshape(H2, 125, KT * T).astype(bf16)
    # packed param planes (see _build)
    adjb = adjT.astype(bf16)    # (500, 500): k-tile rows k*125..
    avb = np.asarray(inputs["actgen_v"], f32).astype(bf16)
    dclb = dclT.astype(bf16)    # (500, 50)
    pbf = np.concatenate(
        [np.concatenate([adjb[k * 125:(k + 1) * 125] for k in range(KT)], axis=1),
         np.concatenate([avb[k * 125:(k + 1) * 125] for k in range(KT)], axis=1),
         np.concatenate([dclb[k * 125:(k + 1) * 125] for k in range(KT)], axis=1)],
        axis=1)                 # (125, 8*500 + 4*50)
    w1f = np.asarray(inputs["gc1_w"], f32)
    pxb = np.concatenate(
        [np.concatenate([xT[k * 126:(k + 1) * 126] for k in range(KT)], axis=1),
         np.concatenate([w1f[k * 126:(k + 1) * 126] for k in range(KT)], axis=1)],
        axis=1).astype(bf16)    # (126, 4*500 + 4*32)
    ohm = np.zeros((R, R * 128), bf16)
    for r in range(R):
        ohm[r, r * 128:r * 128 + SP] = bf16(1.0)
    common = {
        "pbf": np.ascontiguousarray(pbf),
        "pxb": np.ascontiguousarray(pxb),
        "oh": ohm,
    }
    w2f = np.asarray(inputs["gc2_w"], f32)
    b2f = np.asarray(inputs["gc2_b"], f32).reshape(-1)
    b1f = np.asarray(inputs["gc1_b"], f32).reshape(-1)
    u = np.asarray(inputs["gumbel_u"], f32)              # (1000, 50, 500)
    in_maps = []
    for i in range(NCORES):
        m = dict(common)
        # this core owns gc2 output channels (2i, 2i+1); only those 2
        # columns of w2 (and entries of b2) are shipped/computed
        own = [2 * i, 2 * i + 1]
        m["w2"] = np.ascontiguousarray(w2f[:, own])
        m["rows"] = np.concatenate(
            [b1f, b2f[own], gamma, beta]).reshape(1, -1).astype(bf16)
        # [125, 2*KT*T]: per partition, the 2 owned channels' 8KB runs
        # back to back = one 16KB-contiguous DMA descriptor
        m["wr"] = np.ascontiguousarray(
            wr_all[2 * i:2 * i + 2].transpose(1, 0, 2).reshape(125, -1))
        m["u"] = np.ascontiguousarray(u[i * SP:(i + 1) * SP])  # (125, 50, 500)
        in_maps.append(m)
    return in_maps


def run(inputs, trace=False):
    nc = _get_nc()
    in_maps = prep_in_maps(inputs)
    res = bass_utils.run_bass_kernel_spmd(
        nc, in_maps, core_ids=list(range(NCORES)), trace=trace)
    full = np.concatenate(
        [np.asarray(res.results[i]["out"]).astype(np.float32)
         for i in range(NCORES)], axis=0)                # (1000, 50, 500)
    return full, res


def kernel(**inputs):
    full, _ = run(inputs)
    return full
